# revision 6
# baseline (speedup 1.0000x reference)
"""GQA kernel for 8 TRN2 NeuronCores (Bass/Tile, SPMD).

Sharding: core c <-> (b = c//4, g = c%4). Each core computes, for its batch b
and KV group g (4 query heads), the full attention over S=2048 plus its
row-shard of the output projection (features g*256:(g+1)*256 of Wo). The 4
partial o-projections per batch are summed on the host (gather/unshard step).

Device layout notes:
- everything is kept transposed ([feature, seq]) so matmul free dims are >=256
  and fp32r runs at full rate; all matmul operands are native float32r.
- scores are computed transposed (k on partitions, q on free) so softmax needs
  no cross-partition reduction: exp on ScalarE, row-sums via an extra ones
  column appended to V (PSUM accumulates them for free), normalization via a
  rank-1 PE broadcast + vector multiply.
- causal masking is done by bounding the k loop per 256-wide q superblock and
  multiplying the two diagonal 128x256 blocks with precomputed 0/1 masks.
"""

import numpy as np

B, S, H = 2, 2048, 1024
NH, G, HD = 16, 4, 64
HPG = NH // G          # 4 heads per group
GD = HPG * HD          # 256 features per group
N_CORES = 8
SB = 256               # q superblock width
N_SB = S // SB         # 8
KB = 128               # k block
N_KB = S // KB         # 16
VS = HD + 1            # V' slot width (ones column appended)
SCALE = 1.0 / 8.0

_built = {}


def _build(causal: bool):
    """Build + compile the SPMD Bass program. Returns the Bacc object."""
    import concourse.mybir as mybir
    import concourse.tile as tile
    from concourse import bacc

    dt = mybir.dt
    f32 = dt.float32
    f32r = dt.float32r
    EXP = mybir.ActivationFunctionType.Exp

    nc = bacc.Bacc("TRN2", target_bir_lowering=False, debug=False,
                   num_devices=N_CORES)
    xT = nc.dram_tensor("xT", [H, S], f32r, kind="ExternalInput").ap()
    wq = nc.dram_tensor("wq", [H, GD], f32r, kind="ExternalInput").ap()
    wkv = nc.dram_tensor("wkv", [H, 2 * HD], f32r, kind="ExternalInput").ap()
    wo = nc.dram_tensor("wo", [GD, H], f32r, kind="ExternalInput").ap()
    bq = nc.dram_tensor("bq", [GD, 1], f32, kind="ExternalInput").ap()
    bkv = nc.dram_tensor("bkv", [2 * HD, 1], f32, kind="ExternalInput").ap()
    msk = nc.dram_tensor("msk", [KB, 2 * SB], f32, kind="ExternalInput").ap()
    idm = nc.dram_tensor("idm", [128, 64], f32r, kind="ExternalInput").ap()
    onesr = nc.dram_tensor("onesr", [128, 64], f32r, kind="ExternalInput").ap()
    vone = nc.dram_tensor("vone", [128, N_KB * VS], f32r,
                          kind="ExternalInput").ap()
    part = nc.dram_tensor("part", [H, S], f32, kind="ExternalOutput").ap()

    with tile.TileContext(nc) as tc:
        with tc.tile_pool(name="const", bufs=1) as cp, \
             tc.tile_pool(name="mm", bufs=3, space="PSUM") as mmp, \
             tc.tile_pool(name="pv", bufs=2, space="PSUM") as pvp_pool, \
             tc.tile_pool(name="bc", bufs=2, space="PSUM") as bcp, \
             tc.tile_pool(name="pt", bufs=4) as ptp, \
             tc.tile_pool(name="st", bufs=2) as stp, \
             tc.tile_pool(name="sm", bufs=4) as smp:

            idm_t = cp.tile([128, 64], f32r, tag="idm")
            nc.sync.dma_start(idm_t[:], idm[:])
            ones_t = cp.tile([128, 64], f32r, tag="ones")
            nc.sync.dma_start(ones_t[:], onesr[:])
            mask_t = cp.tile([KB, 2 * SB], f32, tag="mask")
            nc.sync.dma_start(mask_t[:], msk[:])

            bq_t = cp.tile([128, 2], f32, tag="bq")
            for m in range(2):
                nc.sync.dma_start(bq_t[:, m:m + 1], bq[m * 128:(m + 1) * 128, :])
            bkv_t = cp.tile([128, 1], f32, tag="bkv")
            nc.sync.dma_start(bkv_t[:], bkv[:])

            xT_t = []
            for kc in range(8):
                t = cp.tile([128, S], f32r, tag=f"xT{kc}", name=f"xT{kc}")
                nc.sync.dma_start(t[:], xT[kc * 128:(kc + 1) * 128, :])
                xT_t.append(t)
            wq_t = []
            for kc in range(8):
                t = cp.tile([128, GD], f32r, tag=f"wq{kc}", name=f"wq{kc}")
                nc.sync.dma_start(t[:], wq[kc * 128:(kc + 1) * 128, :])
                wq_t.append(t)
            wkv_t = []
            for kc in range(8):
                t = cp.tile([128, 2 * HD], f32r, tag=f"wkv{kc}", name=f"wkv{kc}")
                nc.sync.dma_start(t[:], wkv[kc * 128:(kc + 1) * 128, :])
                wkv_t.append(t)
            wo_t = []
            for kc in range(2):
                t = cp.tile([128, H], f32r, tag=f"wo{kc}", name=f"wo{kc}")
                nc.sync.dma_start(t[:], wo[kc * 128:(kc + 1) * 128, :])
                wo_t.append(t)

            # ---- projections: Q^T [256, S], [K^T; V^T] [128, S] ----
            qT_t = [cp.tile([128, S], f32r, tag=f"qT{m}", name=f"qT{m}")
                    for m in range(2)]
            kvT_t = cp.tile([128, S], f32r, tag="kvT")
            for m in range(2):
                for sb in range(N_SB):
                    ps = mmp.tile([128, SB], f32, tag="mm", name="mm_ps")
                    for kc in range(8):
                        nc.tensor.matmul(
                            ps[:], wq_t[kc][:, m * 128:(m + 1) * 128],
                            xT_t[kc][:, sb * SB:(sb + 1) * SB],
                            start=(kc == 0), stop=(kc == 7))
                    nc.vector.tensor_scalar_add(
                        qT_t[m][:, sb * SB:(sb + 1) * SB], ps[:], bq_t[:, m:m + 1])
            for sb in range(N_SB):
                ps = mmp.tile([128, SB], f32, tag="mm", name="mm_ps")
                for kc in range(8):
                    nc.tensor.matmul(
                        ps[:], wkv_t[kc][:],
                        xT_t[kc][:, sb * SB:(sb + 1) * SB],
                        start=(kc == 0), stop=(kc == 7))
                nc.vector.tensor_scalar_add(
                    kvT_t[:, sb * SB:(sb + 1) * SB], ps[:], bkv_t[:, 0:1])

            kdup = cp.tile([128, S], f32r, tag="kdup")
            nc.sync.dma_start(kdup[64:128, :], kvT_t[0:64, :])

            # ---- V' [128, N_KB*VS]: V natural layout + ones column ----
            vt = cp.tile([128, N_KB * VS], f32r, tag="vt")
            nc.sync.dma_start(vt[:], vone[:])
            for sc in range(N_KB):
                tp = mmp.tile([128, SB], f32, tag="mm", name="mm_tp")
                nc.tensor.matmul(
                    tp[0:128, 0:64], kvT_t[64:128, sc * 128:(sc + 1) * 128],
                    idm_t[64:128, :], start=True, stop=True)
                nc.vector.tensor_copy(vt[:, sc * VS:sc * VS + HD], tp[0:128, 0:64])

            # ---- attention ----
            attnT_t = [cp.tile([128, S], f32r, tag=f"attnT{m}", name=f"attnT{m}")
                       for m in range(2)]
            for h in range(HPG):
                qtile = qT_t[h // 2]
                qrow = (h % 2) * 64
                for sb in range(N_SB):
                    nkb = (2 * sb + 2) if causal else N_KB
                    pvp = pvp_pool.tile([128, SB], f32, tag="pv", name="pv_acc")
                    for kb in range(nkb):
                        qk = mmp.tile([128, SB], f32, tag="mm", name="mm_qk")
                        kt = (kvT_t if qrow == 0 else kdup)
                        nc.tensor.matmul(
                            qk[:], kt[qrow:qrow + 64, kb * 128:(kb + 1) * 128],
                            qtile[qrow:qrow + 64, sb * SB:(sb + 1) * SB],
                            start=True, stop=True)
                        pt = ptp.tile([128, SB], f32r, tag="pt", name="pt_t")
                        nc.scalar.activation(pt[:], qk[:], EXP, scale=SCALE)
                        if causal and kb >= 2 * sb:
                            half = kb - 2 * sb
                            nc.vector.tensor_mul(
                                pt[:], pt[:], mask_t[:, half * SB:(half + 1) * SB])
                        nc.tensor.matmul(
                            pvp[0:VS, :], vt[:, kb * VS:(kb + 1) * VS], pt[:],
                            start=(kb == 0), stop=(kb == nkb - 1))
                    rc = smp.tile([65, SB], f32, tag="rc", name="rc_t")
                    nc.vector.reciprocal(rc[64:65, :], pvp[HD:HD + 1, :])
                    rcr = smp.tile([65, SB], f32r, tag="rcr", name="rcr_t")
                    nc.vector.tensor_copy(rcr[64:65, :], rc[64:65, :])
                    bc = bcp.tile([64, SB], f32, tag="bc", name="bc_t")
                    nc.tensor.matmul(bc[:], ones_t[64:65, :], rcr[64:65, :],
                                     start=True, stop=True)
                    bcs = smp.tile([64, SB], f32, tag="bcs", name="bcs_t")
                    nc.vector.tensor_copy(bcs[:], bc[:])
                    if qrow == 0:
                        nc.vector.tensor_mul(
                            attnT_t[h // 2][0:64, sb * SB:(sb + 1) * SB],
                            pvp[0:HD, :], bcs[:])
                    else:
                        ns = smp.tile([64, SB], f32r, tag="ns", name="ns_t")
                        nc.vector.tensor_mul(ns[:], pvp[0:HD, :], bcs[:])
                        nc.sync.dma_start(
                            attnT_t[h // 2][64:128, sb * SB:(sb + 1) * SB], ns[:])

            # ---- o_proj partial: part[m*128:+128, :] = Wo_g^T @ attnT ----
            for m in range(8):
                stage = stp.tile([128, S], f32, tag="st", name="stage_t")
                for sb in range(N_SB):
                    ps = mmp.tile([128, SB], f32, tag="mm", name="mm_ps")
                    for kc in range(2):
                        nc.tensor.matmul(
                            ps[:], wo_t[kc][:, m * 128:(m + 1) * 128],
                            attnT_t[kc][:, sb * SB:(sb + 1) * SB],
                            start=(kc == 0), stop=(kc == 1))
                    nc.vector.tensor_copy(stage[:, sb * SB:(sb + 1) * SB], ps[:])
                nc.sync.dma_start(part[m * 128:(m + 1) * 128, :], stage[:])

    nc.compile()
    return nc


def _get(causal: bool):
    if causal not in _built:
        _built[causal] = _build(causal)
    return _built[causal]


def _make_masks():
    p = np.arange(KB)[:, None]
    t = np.arange(SB)[None, :]
    a = (p <= t).astype(np.float32)
    b = (p + 128 <= t).astype(np.float32)
    return np.concatenate([a, b], axis=1)


def _in_maps(hidden, Wq, bq, Wk, bk, Wv, bv, Wo):
    msk = _make_masks()
    idm = np.zeros((128, 64), np.float32)
    idm[64:128, :] = np.eye(64, dtype=np.float32)
    onesr = np.ones((128, 64), np.float32)
    vone = np.ones((128, N_KB * VS), np.float32)
    maps = []
    for c in range(N_CORES):
        b, g = divmod(c, G)
        maps.append({
            "xT": np.ascontiguousarray(hidden[b].T),
            "wq": np.ascontiguousarray(Wq[:, g * GD:(g + 1) * GD]),
            "wkv": np.ascontiguousarray(
                np.concatenate([Wk[:, g * HD:(g + 1) * HD],
                                Wv[:, g * HD:(g + 1) * HD]], axis=1)),
            "wo": np.ascontiguousarray(Wo[g * GD:(g + 1) * GD, :]),
            "bq": np.ascontiguousarray(bq[g * GD:(g + 1) * GD, None]),
            "bkv": np.ascontiguousarray(
                np.concatenate([bk[g * HD:(g + 1) * HD],
                                bv[g * HD:(g + 1) * HD]])[:, None]),
            "msk": msk,
            "idm": idm,
            "onesr": onesr,
            "vone": vone,
        })
    return maps


def _assemble(results, bo):
    out = np.empty((B, S, H), np.float32)
    for b in range(B):
        oT = results[4 * b]["part"].astype(np.float32).copy()
        for g in range(1, G):
            oT += results[4 * b + g]["part"]
        out[b] = oT.T + bo[None, :]
    return out


def _mask_kind(mask):
    m0 = mask[0, 0]
    if not np.array_equal(mask, np.broadcast_to(m0, mask.shape)):
        return "other"
    if m0.all():
        return "full"
    tri = np.tril(np.ones((S, S), bool))
    if np.array_equal(m0, tri):
        return "causal"
    return "other"


def _numpy_fallback(hidden, mask, Wq, bq, Wk, bk, Wv, bv, Wo, bo):
    q = (hidden @ Wq + bq).reshape(B, S, G, HPG, HD)
    k = (hidden @ Wk + bk).reshape(B, S, G, HD)
    v = (hidden @ Wv + bv).reshape(B, S, G, HD)
    attn = np.empty((B, S, G, HPG, HD), np.float32)
    for b in range(B):
        for g in range(G):
            for h in range(HPG):
                s = (q[b, :, g, h] @ k[b, :, g].T) * SCALE
                s = np.where(mask[b, g * HPG + h], s, -np.inf)
                s -= s.max(axis=-1, keepdims=True)
                np.exp(s, out=s)
                s /= s.sum(axis=-1, keepdims=True)
                attn[b, :, g, h] = s @ v[b, :, g]
    return attn.reshape(B, S, H) @ Wo + bo


def kernel(hidden_state, attention_mask, Wq, bq, Wk, bk, Wv, bv, Wo, bo):
    from concourse.bass_utils import run_bass_kernel_spmd

    hidden = np.asarray(hidden_state, np.float32)
    mask = np.asarray(attention_mask)
    Wq, bq = np.asarray(Wq, np.float32), np.asarray(bq, np.float32)
    Wk, bk = np.asarray(Wk, np.float32), np.asarray(bk, np.float32)
    Wv, bv = np.asarray(Wv, np.float32), np.asarray(bv, np.float32)
    Wo, bo = np.asarray(Wo, np.float32), np.asarray(bo, np.float32)

    kind = _mask_kind(mask)
    if kind == "other":
        return _numpy_fallback(hidden, mask, Wq, bq, Wk, bk, Wv, bv, Wo, bo)

    nc = _get(kind == "causal")
    maps = _in_maps(hidden, Wq, bq, Wk, bk, Wv, bv, Wo)
    res = run_bass_kernel_spmd(nc, maps, core_ids=list(range(N_CORES)))
    return _assemble(res.results, bo)
